# revision 6
# baseline (speedup 1.0000x reference)
"""Trainium2 Bass kernel for nn_CandidateFinder (LSH hash-equality KNN).

Reference semantics: q/k binarized (x>0), projected by W [64,8], sign bits
packed into an 8-bit bucket code; for each query, return the first 64 key
indices (ascending) whose code equals the query's code, padded with -1.

Key insight: codes live in [0,256). Build, per batch, a per-bucket table of
the first 64 key indices, then gather per query. Both steps map onto
matmuls + a free-dim prefix scan + GPSIMD local_scatters.

Sharding: 8 cores = 4 batches x 2 bucket-halves (c in [0,128) / [128,256)).
Each core computes a partial gather (zero where the query's code is in the
other half); host sums the pair and subtracts 1 (table stores j+1, empty=0).

v2 changes vs the 14026ns baseline:
  - inputs host-prestacked to [128, 512] tiles (keys/queries chunk pairs on
    partition blocks 0:64/64:128) so every input DMA hits the 500ns floor
    and each bin is one [128,512] op instead of two.
  - kT halves ride the SP and ACT HWDGE queues (both land ~2.4us); consts
    (sgnc | compact block-diag hash weights | per-bucket bias) are packed
    into one [128,162] fp16 DMA on the Pool queue.
  - hash lhsT compacted to [128,16] per half (out partitions 0:8/8:16), so
    bits live on psum rows 0:16.
  - the scan->mask->sub chain is shortened: onehot is built as {0,2} via
    Relu(scale=2), the scan state starts at 2 (even states 2+2r), and one
    scalar_tensor_tensor idx=(onehot-1)*state replaces mul+sub: matches get
    unique even slots 4..., non-matches negative (ignored by scatter).
  - table slots are strided (rank r at slot 2+2r); the quarter-table merges
    read the strided slots and emit the compact [128,64] gather table.

Precision: the hash sign test needs ~f32-accurate projections. W is split
as fp16(W) + fp16(W - fp16(W)) and the two fp16 matmuls accumulate in f32
PSUM; representation error ~1e-6 vs hash sign margins ~1e-4 on this data.

k-side agree trick on 0/1 bits: #agreeing bits = pm^T bits + (8 - pop(c))
with pm = +-1 bit pattern of bucket c, so onehot2 = Relu(2*pm^T bits +
2*(1 - pop(c))) in {0,2}. On +-1 signs: onehot2 = Relu(2*pm^T s - 14).
q-side keeps {0,1}: onehot = Relu(pm^T s - 7).
Tables are fp16 (iota data j+1; integers <= 2048 are fp16-exact) because
the Pool engine cannot add int16.
"""

import numpy as np
import ml_dtypes

B, L, D, NH = 4, 2048, 64, 8
KMAX = 64
TABLE_ELEMS = 384   # max even slot 2+2*count; count<=190 covered (90 on this data)
HALF = L // 2
QTR = L // 4

_cache = {}


def _build_program():
    import concourse.bass as bass
    import concourse.mybir as mybir
    from concourse import bacc, tile
    from contextlib import ExitStack

    dt = mybir.dt
    Alu = mybir.AluOpType
    Act = mybir.ActivationFunctionType

    nc = bacc.Bacc("TRN2", target_bir_lowering=False, debug=False)

    # DRAM I/O (per-core shapes); kTgs/qTgs are host-prestacked [128, 512]:
    # rows 0:64 = chunk 2g (bf16 x^T), rows 64:128 = chunk 2g+1.
    kT0s_d = nc.declare_dram_parameter("kT0s", [128, QTR], dt.bfloat16, isOutput=False)
    kT1s_d = nc.declare_dram_parameter("kT1s", [128, QTR], dt.bfloat16, isOutput=False)
    qT0s_d = nc.declare_dram_parameter("qT0s", [128, QTR], dt.bfloat16, isOutput=False)
    qT1s_d = nc.declare_dram_parameter("qT1s", [128, QTR], dt.bfloat16, isOutput=False)
    # packed consts: cols 0:128 = pm (+-1 patterns at rows 0:8 and 64:72),
    # 128:200 = hi hash weights (block-diag [128,72]), 200:272 = lo,
    # 272:273 = per-bucket bias 2-2*pop(c), 273:274 = -14, 274:275 = -7,
    # 275:276 pad.
    consts_d = nc.declare_dram_parameter("consts", [128, 276], dt.float16, isOutput=False)
    out_d = nc.declare_dram_parameter("out", [L, KMAX], dt.float16, isOutput=True)

    with ExitStack() as ctx:
        tc = ctx.enter_context(tile.TileContext(nc))
        sb = ctx.enter_context(tc.tile_pool(name="sb", bufs=1))
        hp = ctx.enter_context(tc.tile_pool(name="hp", bufs=3, space="PSUM"))
        ap = ctx.enter_context(tc.tile_pool(name="ap", bufs=3, space="PSUM"))
        gp = ctx.enter_context(tc.tile_pool(name="gp", bufs=1, space="PSUM"))

        # ---- loads: kT halves on the two HWDGE queues, consts on Pool ----
        kT0s_sb = sb.tile([128, QTR], dt.bfloat16, tag="kT0s")
        nc.sync.dma_start(kT0s_sb[:], kT0s_d[:])
        kT1s_sb = sb.tile([128, QTR], dt.bfloat16, tag="kT1s")
        nc.scalar.dma_start(kT1s_sb[:], kT1s_d[:])
        consts_sb = sb.tile([128, 276], dt.float16, tag="consts")
        nc.gpsimd.dma_start(consts_sb[:], consts_d[:])
        qT0s_sb = sb.tile([128, QTR], dt.bfloat16, tag="qT0s")
        nc.sync.dma_start(qT0s_sb[:], qT0s_d[:])
        qT1s_sb = sb.tile([128, QTR], dt.bfloat16, tag="qT1s")
        nc.scalar.dma_start(qT1s_sb[:], qT1s_d[:])

        sgnc = consts_sb[:, 0:128]
        wpk_hi = consts_sb[:, 128:200]
        wpk_lo = consts_sb[:, 200:272]
        biask = consts_sb[:, 272:273]
        bm14 = consts_sb[:, 273:274]
        bm7 = consts_sb[:, 274:275]

        # hash psum tiles; chunk 2g bits land at rows 0:8, chunk 2g+1 at
        # rows 64:72 (matmul SBUF operands need base partition 0/32/64).
        hpk1a = hp.tile([128, 256], dt.float32, tag="hp", name="hpk1a")
        hpk1b = hp.tile([128, 256], dt.float32, tag="hp", name="hpk1b")
        hpk2 = hp.tile([128, 512], dt.float32, tag="hp", name="hpk2")

        # PE warm-up: anchor the p-state clock (a >~3us idle resets the PE
        # ramp). Garbage results land in rows the real hash matmuls
        # overwrite with start=True.
        warm_sb = sb.tile([D, 64], dt.float16, tag="warm")
        nc.vector.memset(warm_sb[:], 0.0)
        for _ in range(2):
            nc.tensor.matmul(
                hpk1a[0:32, 0:64], lhsT=warm_sb[:, 0:32], rhs=warm_sb[:],
                start=True, stop=True,
            )

        def hash_pair(hpt, x2_ap):
            # x2 [128, n]: rows 0:64 = even chunk, 64:128 = odd chunk
            n = x2_ap.shape[-1]
            mm_hi = nc.tensor.matmul(
                hpt[0:72, 0:n], lhsT=wpk_hi, rhs=x2_ap, start=True, stop=False,
            )
            nc.tensor.matmul(
                hpt[0:72, 0:n], lhsT=wpk_lo, rhs=x2_ap, start=False, stop=True,
            )
            return mm_hi

        # ---- k side: bin (DVE, one op per stacked half) -> hash ----
        xk2 = [
            sb.tile([128, QTR], dt.float16, tag=f"xk2{g}", name=f"xk2{g}")
            for g in range(2)
        ]
        nc.vector.tensor_single_scalar(xk2[0][:], kT0s_sb[:], 0.0, Alu.is_gt)
        nc.vector.tensor_single_scalar(xk2[1][:], kT1s_sb[:], 0.0, Alu.is_gt)
        hash_pair(hpk1a, xk2[0][:, 0:256])
        hash_pair(hpk1b, xk2[0][:, 256:512])
        hash_pair(hpk2, xk2[1][:])

        # pair-1 bits as +-1 via ACT Sign in column-halves; pair-2 bits 0/1
        # via DVE is_gt (Relu bias 2-2*popcount)
        s01k = sb.tile([128, 1024], dt.float16, tag="s01k")
        nc.scalar.activation(s01k[0:72, 0:256], hpk1a[0:72, :], Act.Sign)
        nc.scalar.activation(s01k[0:72, 256:512], hpk1b[0:72, :], Act.Sign)
        nc.vector.tensor_single_scalar(s01k[0:72, 512:1024], hpk2[0:72, :], 0.0, Alu.is_gt)

        # ---- q side: bin (Pool, stacked) -> hash -> sign +-1 (ACT) ----
        xq2 = [
            sb.tile([128, QTR], dt.float16, tag=f"xq2{g}", name=f"xq2{g}")
            for g in range(2)
        ]
        nc.gpsimd.tensor_single_scalar(xq2[0][:], qT0s_sb[:], 0.0, Alu.is_gt)
        nc.gpsimd.tensor_single_scalar(xq2[1][:], qT1s_sb[:], 0.0, Alu.is_gt)

        # scatter data: each partition holds 1..L (fp16), off the
        # critical path (scat c0 needs it only after the first scan+stt)
        iota_sb = sb.tile([128, L], dt.float16, tag="iota")
        nc.gpsimd.iota(
            iota_sb[:], pattern=[[1, L]], base=1, channel_multiplier=0,
            allow_small_or_imprecise_dtypes=True,
        )

        onehot = sb.tile([128, L], dt.float16, tag="onehot")
        sq = sb.tile([128, 1024], dt.float16, tag="sq")
        q1h = sb.tile([128, 1536], dt.float16, tag="q1h")
        hpq = [hp.tile([128, 512], dt.float32, tag="hp", name=f"hpq{g}") for g in range(2)]

        def agree(rhs_ap, n, name):
            t = ap.tile([128, n], dt.float32, tag="apt", name=name)
            r = rhs_ap.base_partition()
            mm = nc.tensor.matmul(
                t[:], lhsT=consts_sb[r : r + 8, 0:128],
                rhs=rhs_ap, start=True, stop=True,
            )
            return t, mm

        # PE emission order interleaves the q hash behind the k agrees so
        # the q chain keeps flowing while ACT works on the k relus.
        # k chunk 0 runs in column-halves so the scan chain starts early.
        from concourse.tile_rust import add_dep_helper

        apt_c0a, mm_c0a = agree(s01k[0:8, 0:256], 256, "apt_c0a")
        apt_c0b, mm_c0b = agree(s01k[0:8, 256:512], 256, "apt_c0b")
        apt_c1, mm_c1 = agree(s01k[64:72, 0:512], 512, "apt_c1")
        mm_hq0 = hash_pair(hpq[0], xq2[0][:])
        apt_c2, mm_c2 = agree(s01k[0:8, 512:1024], 512, "apt_c2")
        apt_c3, mm_c3 = agree(s01k[64:72, 512:1024], 512, "apt_c3")
        hash_pair(hpq[1], xq2[1][:])
        # keep the early k agrees (and the scan chain they feed) ahead of
        # everything later on PE
        add_dep_helper(mm_hq0.ins, mm_c1.ins, sync=False,
                       reason="chunk-0/1 agrees before q hash on PE")
        add_dep_helper(mm_c2.ins, mm_c1.ins, sync=False,
                       reason="chunk-0/1 agrees first on PE")

        # k one-hot2 {0,2}: chunks 0,1 from +-1 signs (bias -14, scale 2);
        # 2,3 from 0/1 bits (per-bucket bias 2-2*popcount, scale 2)
        nc.scalar.activation(onehot[:, 0:256], apt_c0a[:], Act.Relu, bias=bm14, scale=2.0)
        nc.scalar.activation(onehot[:, 256:512], apt_c0b[:], Act.Relu, bias=bm14, scale=2.0)
        nc.scalar.activation(onehot[:, 512:1024], apt_c1[:], Act.Relu, bias=bm14, scale=2.0)
        nc.scalar.activation(onehot[:, 1024:1536], apt_c2[:], Act.Relu, bias=biask, scale=2.0)
        nc.scalar.activation(onehot[:, 1536:2048], apt_c3[:], Act.Relu, bias=biask, scale=2.0)
        nc.scalar.activation(sq[0:72, 0:512], hpq[0][0:72, :], Act.Sign)
        nc.scalar.activation(sq[0:72, 512:1024], hpq[1][0:72, :], Act.Sign)

        aptq = {}
        for u in range(4):
            r = 64 * (u % 2)
            g = u // 2
            aptq[u], _ = agree(sq[r : r + 8, 512 * g : 512 * (g + 1)], 512, f"aptq{u}")
        # q one-hot {0,1}: chunks 0-2 on ACT, chunk 3 on DVE (after the scan chain)
        for u in range(3):
            nc.scalar.activation(
                q1h[:, 512 * u : 512 * (u + 1)], aptq[u][:],
                Act.Relu, bias=bm7,
            )

        # ---- rank keys within bucket: scan state = 2*(1+count) (even,
        # unique at matches); idx = (onehot2-1)*state is the even slot at
        # matches, negative (ignored) elsewhere. One stt replaces mul+sub.
        rank = sb.tile([128, L], dt.float16, tag="rank")
        idx16 = sb.tile([128, L], dt.int16, tag="idx16")
        pieces = [(0, 256), (256, 512), (512, 1024), (1024, 1536), (1536, 2048)]
        stt_inst = {}
        for i, (lo, hi) in enumerate(pieces):
            init = 2.0 if lo == 0 else rank[:, lo - 1 : lo]
            nc.vector.tensor_tensor_scan(
                rank[:, lo:hi], onehot[:, lo:hi], onehot[:, lo:hi],
                init, Alu.add, Alu.bypass,
            )
            stt_inst[i] = nc.vector.scalar_tensor_tensor(
                idx16[:, lo:hi], onehot[:, lo:hi], 1.0, rank[:, lo:hi],
                Alu.subtract, Alu.mult,
            )
        tabs = []
        for c in range(4):
            lo, hi = QTR * c, QTR * (c + 1)
            tab = sb.tile([128, TABLE_ELEMS], dt.float16, tag=f"table{c}")
            tabs.append(tab)
            nc.gpsimd.local_scatter(
                tab[:], iota_sb[:, lo:hi], idx16[:, lo:hi],
                channels=128, num_elems=TABLE_ELEMS, num_idxs=QTR,
            )

        # q one-hot chunk 3 on DVE, held behind the scan chain so it does
        # not preempt the table build
        q1hx = sb.tile([128, 1024], dt.float16, tag="q1hx")
        aptq3_bf = aptq[3][:].bitcast(dt.bfloat16)
        q3_inst = nc.vector.tensor_single_scalar(q1hx[:], aptq3_bf, 7.0, Alu.is_gt)
        add_dep_helper(
            q3_inst.ins, stt_inst[4].ins, sync=False,
            reason="finish scan chain before q one-hot tail",
        )

        # merge quarter tables on Pool (disjoint nonzero slots); strided
        # slots 4,6,..,130 hold matches 1..64 (j+1) per bucket
        def slot64(t):
            return t[:, 4 : 4 + 2 * KMAX].rearrange("c (s two) -> c s two", two=2)[:, :, 0]

        m01 = sb.tile([128, KMAX], dt.float16, tag="m01")
        nc.gpsimd.tensor_add(m01[:], slot64(tabs[0]), slot64(tabs[1]))
        m23 = sb.tile([128, KMAX], dt.float16, tag="m23")
        nc.gpsimd.tensor_add(m23[:], slot64(tabs[2]), slot64(tabs[3]))
        tab16 = sb.tile([128, KMAX], dt.float16, tag="tab16")
        nc.gpsimd.tensor_add(tab16[:], m01[:], m23[:])

        # ---- gather per query: out[i, s] = sum_c q1h[c, i] * tab16[c, s] ----
        # Chunk t takes queries 128t..128t+128, so psum partition p holds
        # query 128t+p -> contiguous per-partition DRAM rows (host unpermutes).
        q1hx_v = q1hx[:].rearrange("c (i two) -> c i two", two=2)[:, :, 1]
        HO = 8 * KMAX
        opA = gp.tile([128, HO], dt.float32, tag="gather", name="opA")
        opB = gp.tile([128, HO], dt.float32, tag="gatherB", name="opB")
        for t in range(16):
            dst = opA if t < 8 else opB
            if t < 12:
                lhsT = q1h[:, 128 * t : 128 * (t + 1)]
            else:
                lhsT = q1hx_v[:, 128 * (t - 12) : 128 * (t - 11)]
            nc.tensor.matmul(
                dst[:, KMAX * (t % 8) : KMAX * (t % 8 + 1)],
                lhsT=lhsT, rhs=tab16[:],
                start=True, stop=True,
            )
        out_v = out_d[:].rearrange("(p t) s -> p (t s)", p=128)  # [128, 1024] row-major view
        out0_sb = sb.tile([128, HO], dt.float16, tag="out0_sb")
        nc.vector.tensor_copy(out0_sb[:], opA[:])
        nc.sync.dma_start(out_v[:, 0:HO], out0_sb[:])
        out1_sb = sb.tile([128, HO], dt.float16, tag="out1_sb")
        nc.scalar.activation(out1_sb[:], opB[:], Act.Copy)
        nc.scalar.dma_start(out_v[:, HO : 2 * HO], out1_sb[:])

    nc.compile()
    return nc


def _get_nc():
    if "nc" not in _cache:
        _cache["nc"] = _build_program()
    return _cache["nc"]


def _make_in_maps(query, key, W):
    query = np.asarray(query, dtype=np.float32)
    key = np.asarray(key, dtype=np.float32)
    W = np.asarray(W, dtype=np.float32)

    def stack_half(xT, g):
        # [64, 2048] -> [128, 512]: rows 0:64 = chunk 2g, 64:128 = chunk 2g+1
        a = xT[:, 1024 * g : 1024 * g + 512]
        b = xT[:, 1024 * g + 512 : 1024 * (g + 1)]
        return np.ascontiguousarray(np.concatenate([a, b], axis=0))

    qs, ks = [], []
    for b in range(B):
        qT = query[b].T.astype(ml_dtypes.bfloat16)
        kT = key[b].T.astype(ml_dtypes.bfloat16)
        qs.append([stack_half(qT, 0), stack_half(qT, 1)])
        ks.append([stack_half(kT, 0), stack_half(kT, 1)])

    whi = W.astype(np.float16)
    wlo = (W - whi.astype(np.float32)).astype(np.float16)
    # compact block-diagonal pair weights [128, 144]: cols 0:8 map rows
    # 0:64 (even chunk) to psum rows 0:8; cols 64:72 map rows 64:128 to
    # psum rows 64:72.
    wpk = np.zeros((128, 144), np.float16)
    wpk[0:D, 0:NH] = whi
    wpk[D : 2 * D, D : D + NH] = whi
    wpk[0:D, 72 : 72 + NH] = wlo
    wpk[D : 2 * D, 72 + D : 72 + D + NH] = wlo

    consts = []
    for h in range(2):
        cg = 128 * h + np.arange(128)  # global bucket ids of this half
        bits = ((cg[None, :] >> np.arange(NH)[:, None]) & 1).astype(np.float32)
        pm = (2.0 * bits - 1.0).astype(np.float16)  # [8, 128]
        arr = np.zeros((128, 276), np.float16)
        arr[0:NH, 0:128] = pm
        arr[D : D + NH, 0:128] = pm
        arr[:, 128:272] = wpk
        arr[:, 272] = (2.0 - 2.0 * bits.sum(axis=0)).astype(np.float16)
        arr[:, 273] = -14.0
        arr[:, 274] = -7.0
        consts.append(arr)
    return [
        {
            "kT0s": ks[c // 2][0],
            "kT1s": ks[c // 2][1],
            "qT0s": qs[c // 2][0],
            "qT1s": qs[c // 2][1],
            "consts": consts[c % 2],
        }
        for c in range(2 * B)
    ]


def _combine(results):
    # device layout: [128, 16*64], partition p col t*64+s <-> query 128t+p
    out = np.empty((B, L, KMAX), dtype=np.int64)
    for b in range(B):
        g = results[2 * b]["out"].astype(np.int64) + results[2 * b + 1]["out"].astype(
            np.int64
        )
        g = g.reshape(128, 16, KMAX).transpose(1, 0, 2).reshape(L, KMAX)
        out[b] = g - 1
    return out


def _run_spmd(in_maps, **kwargs):
    from concourse.bass_utils import run_bass_kernel_spmd

    return run_bass_kernel_spmd(_get_nc(), in_maps, list(range(2 * B)), **kwargs)


def kernel(query, key, W, head_idx=0, **_unused):
    in_maps = _make_in_maps(query, key, W)
    res = _run_spmd(in_maps)
    return _combine(res.results)
